# revision 61
# baseline (speedup 1.0000x reference)
"""Trainium2 Bass kernel for the pre-norm transformer block (nn_Block_54511724920843).

Sharding: data-parallel over the batch axis b (8 groups of 1024 tokens, one
per NeuronCore). Weights replicated. BatchNorm statistics span all 8192
tokens; per-core (sum, sumsq) are combined with a small AllGather + local sum
(cheaper than AllReduce in the collective cost model), once per BN.

On-chip layout is feature-major ("transposed"): activations are [feature,
token] so projections take the natural weight matrices as lhsT and
per-feature BN scale/shift become per-partition scalars.

Attention (the Act engine's 55us of exp is the bottleneck, so everything
else hides under it): QKV projection emission is interleaved with the head
loop; scoresT[k, q] matmuls run in f32r; exp (with additive mask + 1/8 scale
fused, output prescaled by 1/16 via a -4ln2 bias so fp8e4m3 cannot clip)
writes fp8 chunk pairs; AV contracts them against fp8 v in DoubleRow mode
(0.5 cycles/row). v_aug column 0 holds ones so softmax denominators land in
PSUM partition 0 (HW partition_broadcast only reads partition 0). Head pairs
pack into 128 partitions for the output projection, which also runs fp8
DoubleRow (oT2 x16 / Wo x8 prescale, /128 folded into the evacuation scale).
fp8 is used only where quantization noise is damped at the output (the
attention branch contributes ~3% of the residual stream); the MLP runs in
bf16 - fp8 there costs ~2-3% output error, over the 2e-2 budget.

Activation-table loads (~1.3-2.7us each) are pinned into idle Act windows by
chaining warm-up activations onto pipeline-stage outputs.
"""
import sys

sys.path.insert(0, "/opt/trn_rl_repo")

import numpy as np
import ml_dtypes

import concourse.bass as bass
import concourse.tile as tile
import concourse.mybir as mybir
from concourse import bacc
from concourse.bass_utils import run_bass_kernel_spmd

F32 = mybir.dt.float32
F32R = mybir.dt.float32r
BF16 = mybir.dt.bfloat16
FP8 = mybir.dt.float8e4
DR = mybir.MatmulPerfMode.DoubleRow
AF = mybir.ActivationFunctionType
ALU = mybir.AluOpType

N_CORES = 8
B, GS, ED = 8, 1024, 512
H = 8
DH = ED // H            # 64
TOK = GS                # tokens per core
NT = ED // 128          # 4 feature tiles
NH = ED * 4 // 128      # 16 hidden tiles
NC_TOK = TOK // 128     # 8 token chunks
EPS = 1e-5
N_TOTAL = B * GS        # 8192


def _bn_partial(nc, pools, name, t, x_tile, local):
    """Per-tile local (sum, sumsq) into local[:, 2t:2t+2]."""
    small, statsp, dram = pools
    st = small.tile([128, 2, 6], F32, tag=f"st_{name}", name=f"st_{name}")
    nc.vector.bn_stats(out=st[:, 0, :], in_=x_tile[:, 0:512])
    nc.vector.bn_stats(out=st[:, 1, :], in_=x_tile[:, 512:1024])
    mv = small.tile([128, 2], F32, tag=f"mv_{name}", name=f"mv_{name}")
    nc.vector.bn_aggr(out=mv, in_=st)
    nc.vector.tensor_scalar(
        out=local[:, 2 * t : 2 * t + 1], in0=mv[:, 0:1],
        scalar1=float(TOK), scalar2=None, op0=ALU.mult)
    msq = small.tile([128, 1], F32, tag=f"msq_{name}", name=f"msq_{name}")
    nc.vector.tensor_mul(out=msq, in0=mv[:, 0:1], in1=mv[:, 0:1])
    vps = small.tile([128, 1], F32, tag=f"vps_{name}", name=f"vps_{name}")
    nc.vector.tensor_add(out=vps, in0=mv[:, 1:2], in1=msq)
    nc.vector.tensor_scalar(
        out=local[:, 2 * t + 1 : 2 * t + 2], in0=vps,
        scalar1=float(TOK), scalar2=None, op0=ALU.mult)


def _bn_device(nc, tc, pools, x_tiles, g_sb, be_sb, eps_sb, name,
               collectives=True, local=None):
    """Global BatchNorm scale/shift from per-core x_tiles (4x [128,1024] f32).

    Returns (s_all, t_all): [128, 4] f32, per-feature scale and shift such that
    bn(x) = x*s + t. Uses bn_stats locally + AllReduce of (sum, sumsq).
    If ``local`` is given, per-tile stats were already emitted via _bn_partial.
    """
    small, statsp, dram = pools
    if local is None:
        local = statsp.tile([128, 8], F32, tag=f"loc_{name}", name=f"loc_{name}")
        for t in range(NT):
            _bn_partial(nc, pools, name, t, x_tiles[t], local)

    bounce_in = dram.tile([128, 8], F32, tag=f"bi_{name}", name=f"bi_{name}")
    nc.scalar.dma_start(out=bounce_in, in_=local)
    if collectives:
        # AllGather (15.8us model cost) beats AllReduce (28.3us); sum the 8
        # per-core stat blocks locally afterwards (3 halving adds).
        bounce_out = dram.tile([N_CORES, 128, 8], F32, tag=f"bo_{name}",
                               name=f"bo_{name}")
        nc.gpsimd.collective_compute(
            "AllGather", ALU.bypass,
            replica_groups=[list(range(N_CORES))],
            ins=[bounce_in[:]], outs=[bounce_out[:]])
        gath = statsp.tile([128, N_CORES, 8], F32, tag=f"ga_{name}",
                           name=f"ga_{name}")
        nc.scalar.dma_start(
            out=gath, in_=bounce_out.rearrange("r p s -> p r s"))
        h4 = statsp.tile([128, 4, 8], F32, tag=f"h4_{name}", name=f"h4_{name}")
        nc.vector.tensor_add(out=h4, in0=gath[:, 0:4, :], in1=gath[:, 4:8, :])
        h2t = statsp.tile([128, 2, 8], F32, tag=f"h2_{name}", name=f"h2_{name}")
        nc.vector.tensor_add(out=h2t, in0=h4[:, 0:2, :], in1=h4[:, 2:4, :])
        glob = statsp.tile([128, 8], F32, tag=f"gl_{name}", name=f"gl_{name}")
        nc.vector.tensor_add(out=glob, in0=h2t[:, 0, :], in1=h2t[:, 1, :])
    else:
        bounce_out = dram.tile([128, 8], F32, tag=f"bo_{name}",
                               name=f"bo_{name}")
        nc.scalar.dma_start(out=bounce_out, in_=bounce_in[:])
        glob = statsp.tile([128, 8], F32, tag=f"gl_{name}", name=f"gl_{name}")
        nc.scalar.dma_start(out=glob, in_=bounce_out)
        nc.vector.tensor_scalar(out=glob, in0=glob, scalar1=float(N_CORES),
                                scalar2=None, op0=ALU.mult)

    s_all = statsp.tile([128, 4], F32, tag=f"s_{name}", name=f"s_{name}")
    t_all = statsp.tile([128, 4], F32, tag=f"t_{name}", name=f"t_{name}")
    inv_n = 1.0 / float(N_TOTAL)
    gv = glob.rearrange("p (t two) -> p two t", two=2)
    sums, sqs = gv[:, 0, :], gv[:, 1, :]
    m = small.tile([128, 4], F32, tag=f"m_{name}", name=f"m_{name}")
    nc.vector.tensor_scalar(out=m, in0=sums, scalar1=inv_n, scalar2=None,
                            op0=ALU.mult)
    msq = small.tile([128, 4], F32, tag=f"gmsq_{name}", name=f"gmsq_{name}")
    nc.vector.tensor_mul(out=msq, in0=m, in1=m)
    # var = sumsq/N - mean^2
    var = small.tile([128, 4], F32, tag=f"var_{name}", name=f"var_{name}")
    nc.vector.scalar_tensor_tensor(
        out=var, in0=sqs, scalar=inv_n, in1=msq, op0=ALU.mult, op1=ALU.subtract)
    # rstd = sqrt(1/(var + eps)); reciprocal on DVE (accurate), Sqrt table prewarmed
    vpe = small.tile([128, 4], F32, tag=f"vpe_{name}", name=f"vpe_{name}")
    nc.vector.tensor_scalar(out=vpe, in0=var, scalar1=EPS, scalar2=None,
                            op0=ALU.add)
    rec = small.tile([128, 4], F32, tag=f"rec_{name}", name=f"rec_{name}")
    nc.vector.reciprocal(out=rec, in_=vpe)
    rstd = small.tile([128, 4], F32, tag=f"rstd_{name}", name=f"rstd_{name}")
    nc.scalar.activation(out=rstd, in_=rec, func=AF.Sqrt)
    # s = g * rstd ; t = be - mean * s
    nc.vector.tensor_mul(out=s_all, in0=g_sb, in1=rstd)
    sn = small.tile([128, 4], F32, tag=f"sn_{name}", name=f"sn_{name}")
    nc.vector.tensor_mul(out=sn, in0=s_all, in1=m)
    nc.vector.tensor_sub(out=t_all, in0=be_sb, in1=sn)
    return s_all, t_all


def build(sim=False, collectives=True, n_devices=N_CORES, stop_after=None,
          repeat=1):
    nc = _build_graph(sim=sim, collectives=collectives, n_devices=n_devices,
                      stop_after=stop_after, repeat=repeat)
    nc.compile()
    return nc


def _build_graph(sim=False, collectives=True, n_devices=N_CORES,
                 stop_after=None, repeat=1):
    from contextlib import ExitStack

    nc = bacc.Bacc("TRN2", target_bir_lowering=False, debug=False,
                   num_devices=n_devices)

    XT = nc.dram_tensor("xt", [NT, 128, TOK], F32, kind="ExternalInput")
    WQ = nc.dram_tensor("wq", [128, NT, 512], F32, kind="ExternalInput")
    WK = nc.dram_tensor("wk", [128, NT, 512], F32, kind="ExternalInput")
    WV = nc.dram_tensor("wv", [128, NT, 512], F32, kind="ExternalInput")
    WO = nc.dram_tensor("wo", [128, H // 2, 512], FP8, kind="ExternalInput")
    WM1 = nc.dram_tensor("wm1", [128, NT, 2048], BF16, kind="ExternalInput")
    WM2 = nc.dram_tensor("wm2", [128, NH, 512], BF16, kind="ExternalInput")
    BQ = nc.dram_tensor("bq", [128, 4], F32, kind="ExternalInput")
    BK = nc.dram_tensor("bk", [128, 4], F32, kind="ExternalInput")
    BV = nc.dram_tensor("bv", [128, 512], F32, kind="ExternalInput")
    BO = nc.dram_tensor("bo", [128, 4], F32, kind="ExternalInput")
    B1 = nc.dram_tensor("b1m", [128, 16], F32, kind="ExternalInput")
    B2 = nc.dram_tensor("b2m", [128, 4], F32, kind="ExternalInput")
    G1 = nc.dram_tensor("g1", [128, 4], F32, kind="ExternalInput")
    BE1 = nc.dram_tensor("be1", [128, 4], F32, kind="ExternalInput")
    G2 = nc.dram_tensor("g2", [128, 4], F32, kind="ExternalInput")
    BE2 = nc.dram_tensor("be2", [128, 4], F32, kind="ExternalInput")
    AM = nc.dram_tensor("am", [128, 8], F32, kind="ExternalInput")
    OUT = nc.dram_tensor("outt", [NT, 128, TOK], F32, kind="ExternalOutput")

    gelu_func = AF.Exp if sim else AF.Gelu

    # NOTE: deliberately shallow 1-space indents below so the 270-line body
    # keeps its original indentation while gaining a repeat loop (used only
    # for marginal HW timing; repeat=1 for the graded kernel).
    with tile.TileContext(nc) as tc:
     for _it in range(repeat):
      with ExitStack() as ctx:
        vec = ctx.enter_context(tc.tile_pool(name="vec", bufs=1))
        small = ctx.enter_context(tc.tile_pool(name="small", bufs=8))
        statsp = ctx.enter_context(tc.tile_pool(name="stats", bufs=1))
        dram = ctx.enter_context(tc.tile_pool(name="dram", bufs=1, space="DRAM"))
        mlpwp = ctx.enter_context(tc.tile_pool(name="mlpw", bufs=1))
        x2p = ctx.enter_context(tc.tile_pool(name="x2", bufs=1))

        def vload(name, dram_t, shape, dtype=F32):
            t = vec.tile(shape, dtype, tag=name, name=name)
            nc.sync.dma_start(out=t, in_=dram_t[:, :])
            return t

        x2_tiles = [x2p.tile([128, TOK], F32, tag=f"x2_{t}", name=f"x2_{t}") for t in range(NT)]

        def dump_out(tiles, cast=False):
            for t in range(NT):
                src_ap = tiles[t].bitcast(F32) if cast else tiles[t]
                nc.sync.dma_start(out=OUT[t, :, :], in_=src_ap)

        with ExitStack() as s1:
            xp = s1.enter_context(tc.tile_pool(name="xt", bufs=1))
            qkp = s1.enter_context(tc.tile_pool(name="qk", bufs=1))
            vap = s1.enter_context(tc.tile_pool(name="vaug", bufs=1))

            # spread the x loads over three DMA queues so bn_stats (and with
            # it the BN1 collective) starts as early as possible
            x_tiles = []
            x_queues = [nc.sync, nc.scalar, nc.gpsimd, nc.sync]
            for t in range(NT):
                xt = xp.tile([128, TOK], F32, tag=f"x_{t}", name=f"x_{t}")
                x_queues[t].dma_start(out=xt, in_=XT[t, :, :])
                x_tiles.append(xt)

            q_tiles = [qkp.tile([128, TOK], F32R, tag=f"q_{t}", name=f"q_{t}") for t in range(NT)]
            k_tiles = [qkp.tile([128, TOK], F32R, tag=f"k_{t}", name=f"k_{t}") for t in range(NT)]
            # v_aug column layout per (chunk, head): col 0 = ones (softmax
            # denominator lands in PSUM partition 0 — HW partition_broadcast
            # only reads partition 0), cols 64:128 = v features. Cols 1:64
            # are zeroed on the idle Pool engine (their PSUM rows 1:64 are
            # never read, but keep the sim deterministic). fp8 so the AV
            # contraction runs in DoubleRow mode (0.5 cycles/row).
            v_aug = vap.tile([128, NC_TOK, H, 128], FP8, tag="vaug", name="vaug")

            # oT2 packs head pairs: partitions 0-63 = head 2i, 64-127 = head
            # 2i+1 (pair index i) so the output projection contracts both
            # heads of a pair in a single 128-deep matmul.
            # fp8 (x16 prescale in the epilogue multiply; Wo carries x8, the
            # combined /128 is undone by the oproj evacuation scale)
            otp = s1.enter_context(tc.tile_pool(name="ot", bufs=1))
            oT2 = otp.tile([128, H // 2, TOK], FP8, tag="ot", name="ots")
            wop = s1.enter_context(tc.tile_pool(name="wo", bufs=1))

            # ======== Phase 1+2: BN1, QKV projections interleaved with
            # attention. Emission order = PE order: (q0,k0), v, then head
            # pairs with the next q/k tile production spliced between pairs,
            # so the first exp starts ~15us earlier and the projection
            # matmuls hide under the Act-bound attention stream. QKV shares
            # the S PSUM slots (scp) — PSUM is exactly 8 banks: scp 2x2 +
            # avp 2x2.
            with ExitStack() as s2:
                wqp = s2.enter_context(tc.tile_pool(name="wqkv", bufs=1))
                h1p = s2.enter_context(tc.tile_pool(name="h1", bufs=1))
                scp = s2.enter_context(
                    tc.tile_pool(name="sc", bufs=2, space="PSUM"))
                avp = s2.enter_context(
                    tc.tile_pool(name="av", bufs=1, space="PSUM"))
                pj = s2.enter_context(
                    tc.tile_pool(name="pj", bufs=2, space="PSUM"))
                ep = s2.enter_context(tc.tile_pool(name="E", bufs=3))
                rp = s2.enter_context(tc.tile_pool(name="rec", bufs=2))

                g1_sb = vload("g1", G1, [128, 4])
                be1_sb = vload("be1", BE1, [128, 4])
                wq_sb = wqp.tile([128, NT, 512], F32R, tag="wq", name="wqs")
                nc.sync.dma_start(out=wq_sb, in_=WQ.bitcast(F32R)[:, :, :])
                wk_sb = wqp.tile([128, NT, 512], F32R, tag="wk", name="wks")
                nc.sync.dma_start(out=wk_sb, in_=WK.bitcast(F32R)[:, :, :])
                wv_sb = wqp.tile([128, NT, 512], F32R, tag="wv", name="wvs")
                nc.sync.dma_start(out=wv_sb, in_=WV.bitcast(F32R)[:, :, :])
                bq_sb = vload("bq", BQ, [128, 4])
                bk_sb = vload("bk", BK, [128, 4])
                bv_sb = vload("bv", BV, [128, 512])
                am_sb = vload("am", AM, [128, 8])
                bo_sb = vload("bo", BO, [128, 4])
                g2_sb = vload("g2", G2, [128, 4])
                be2_sb = vload("be2", BE2, [128, 4])
                b1_sb = vload("b1", B1, [128, 16])
                b2_sb = vload("b2", B2, [128, 4])
                wo_sb = wop.tile([128, H // 2, 512], FP8, tag="wo", name="wos")
                nc.sync.dma_start(out=wo_sb, in_=WO[:, :, :])
                wm1_sb = mlpwp.tile([128, NT, 2048], BF16, tag="wm1",
                                    name="wm1s")
                nc.sync.dma_start(out=wm1_sb, in_=WM1[:, :, :])
                wm2_sb = mlpwp.tile([128, NH, 512], BF16, tag="wm2",
                                    name="wm2s")
                nc.sync.dma_start(out=wm2_sb, in_=WM2[:, :, :])
                eps_sb = vec.tile([128, 1], F32, tag="eps", name="eps")
                nc.vector.memset(eps_sb, EPS)
                # pre-warm the Sqrt table so BN1's rstd needs no load on the
                # post-collective critical path
                warm0 = vec.tile([128, 1], F32, tag="warm0", name="warm0")
                nc.scalar.activation(out=warm0, in_=eps_sb, func=AF.Sqrt)

                s1v, t1v = _bn_device(nc, tc, (small, statsp, dram),
                                      x_tiles, g1_sb, be1_sb, eps_sb, "bn1",
                                      collectives=collectives)
                nc.gpsimd.memset(v_aug[:, :, :, 0:1], 1.0)
                nc.gpsimd.memset(v_aug[:, :, :, 1:64], 0.0)
                if stop_after == "bn1":
                    dump_out(x_tiles)
                    return nc

                # chain the Exp table load right behind the BN1 result so it
                # fills the idle Act window before the first attention exp
                warm = vec.tile([128, 1], F32, tag="warm", name="warm")
                nc.scalar.activation(out=warm, in_=s1v[:, 0:1], func=AF.Exp)

                h1_tiles = []
                for t in range(NT):
                    h1 = h1p.tile([128, TOK], F32R, tag=f"h1_{t}", name=f"h1_{t}")
                    nc.vector.tensor_scalar(
                        out=h1, in0=x_tiles[t],
                        scalar1=s1v[:, t : t + 1], scalar2=t1v[:, t : t + 1],
                        op0=ALU.mult, op1=ALU.add)
                    h1_tiles.append(h1)

                def emit_qk(o, act_evac=False):
                    # q then k tile o; evacuation (bias add) on DVE so the
                    # Act engine stays dedicated to the exp stream. For tile
                    # 0 (before the exp stream starts) split evacuation
                    # across Act+DVE to reach the first score sooner.
                    for (w_sb, b_sb, dst) in ((wq_sb, bq_sb, q_tiles),
                                              (wk_sb, bk_sb, k_tiles)):
                        for hf in range(2):
                            p = pj.tile([128, 512], F32, tag="pj", name="pjs")
                            for k in range(NT):
                                nc.tensor.matmul(
                                    p,
                                    w_sb[:, k, o * 128 : (o + 1) * 128],
                                    h1_tiles[k][:, hf * 512 : (hf + 1) * 512],
                                    start=(k == 0), stop=(k == NT - 1))
                            if act_evac and hf == 0:
                                nc.scalar.activation(
                                    out=dst[o][:, 0:512], in_=p,
                                    func=AF.Identity,
                                    bias=b_sb[:, o : o + 1], scale=1.0)
                            else:
                                nc.vector.tensor_scalar(
                                    out=dst[o][:, hf * 512 : (hf + 1) * 512],
                                    in0=p, scalar1=b_sb[:, o : o + 1],
                                    scalar2=None, op0=ALU.add)

                def emit_v():
                    for tt in range(NC_TOK):
                        p = pj.tile([128, 512], F32, tag="pj", name="pjs")
                        for k in range(NT):
                            nc.tensor.matmul(
                                p,
                                h1_tiles[k][:, tt * 128 : (tt + 1) * 128],
                                wv_sb[:, k, :],
                                start=(k == 0), stop=(k == NT - 1))
                        nc.vector.tensor_add(
                            out=v_aug[:, tt, :, 64:128],
                            in0=p.rearrange("p (h d) -> p h d", h=H),
                            in1=bv_sb.rearrange("p (h d) -> p h d", h=H))

                NP2 = NC_TOK // 2

                def av_pair(av, h, pair, Ep):
                    for hf in range(2):
                        nc.tensor.matmul(
                            av[:, hf * 512 : (hf + 1) * 512],
                            v_aug[:, 2 * pair : 2 * pair + 2, h, :],
                            Ep[:, :, hf * 512 : (hf + 1) * 512],
                            start=(pair == 0), stop=(pair == NP2 - 1),
                            perf_mode=DR)

                last_srec = [None]

                def head(h):
                    t = h // 2
                    r = (h % 2) * 64
                    av = avp.tile([128, TOK], F32, tag="av", name="avs")
                    pend = None
                    Ep = None
                    for c in range(NC_TOK):
                        S = scp.tile([128, TOK], F32, tag="S", name="Ss")
                        for hf in range(2):
                            nc.tensor.matmul(
                                S[:, hf * 512 : (hf + 1) * 512],
                                k_tiles[t][r : r + 64, c * 128 : (c + 1) * 128],
                                q_tiles[t][r : r + 64, hf * 512 : (hf + 1) * 512],
                                start=True, stop=True)
                        if c % 2 == 0:
                            Ep = ep.tile([128, 2, TOK], FP8, tag="E", name="Es")
                        # exp emits E/16 (bias carries -4*ln2) so the fp8 max
                        # of 240 cannot clip the tail; /16 cancels in softmax.
                        nc.scalar.activation(
                            out=Ep[:, c % 2, :], in_=S, func=AF.Exp,
                            bias=am_sb[:, c : c + 1], scale=0.125)
                        if c % 2 == 1:
                            if pend is not None:
                                av_pair(av, h, *pend)
                            pend = (c // 2, Ep)
                    av_pair(av, h, *pend)
                    # softmax denominators live in row 0 of av, head output
                    # in rows 64:128 (both HW-legal partition starts).
                    srec = rp.tile([1, TOK], F32, tag="srec", name="srecs")
                    nc.vector.reciprocal(out=srec, in_=av[0:1, :])
                    last_srec[0] = srec
                    recb = rp.tile([64, TOK], F32, tag="recb", name="recbs")
                    nc.gpsimd.partition_broadcast(recb, srec[0:1, :])
                    nc.vector.scalar_tensor_tensor(
                        out=oT2[r : r + 64, t, :],
                        in0=av[64:128, :], scalar=16.0, in1=recb,
                        op0=ALU.mult, op1=ALU.mult)

                emit_qk(0, act_evac=True)
                emit_v()
                for pair in range(H // 2):
                    if pair > 0:
                        emit_qk(pair)
                    head(2 * pair)
                    head(2 * pair + 1)

                # Sqrt table load for BN2's rstd, pinned behind the last
                # head's reciprocal (idle Act window, before the collective)
                warm2 = vec.tile([1, 1], F32, tag="warm2", name="warm2")
                nc.scalar.activation(out=warm2, in_=last_srec[0][0:1, 0:1],
                                     func=AF.Sqrt)

            if stop_after == "attn":
                dump_out(x_tiles)
                return nc

            # ======== Phase 3: output projection + residual ========
            # Evacuation split across engines so the DVE queue (which also
            # runs bn_stats) isn't the serial bottleneck before the BN2
            # collective: Act does (proj + bo) out of PSUM, Pool adds the
            # residual, DVE only does stats.
            with ExitStack() as s4:
                pop = s4.enter_context(
                    tc.tile_pool(name="po", bufs=4, space="PSUM"))
                otmp = s4.enter_context(tc.tile_pool(name="otmp", bufs=3))
                bn2_local = statsp.tile([128, 8], F32, tag="loc_bn2",
                                        name="loc_bn2")
                for o in range(NT):
                    for hf in range(2):
                        p = pop.tile([128, 512], F32, tag="po", name="pos")
                        for j in range(H // 4):
                            nc.tensor.matmul(
                                p,
                                wo_sb[:, 2 * j : 2 * j + 2,
                                      o * 128 : (o + 1) * 128],
                                oT2[:, 2 * j : 2 * j + 2,
                                    hf * 512 : (hf + 1) * 512],
                                start=(j == 0), stop=(j == H // 4 - 1),
                                perf_mode=DR)
                        tmp = otmp.tile([128, 512], F32, tag="otmp",
                                        name="otmps")
                        nc.scalar.activation(
                            out=tmp, in_=p, func=AF.Identity,
                            bias=bo_sb[:, o : o + 1], scale=1.0 / 128.0)
                        nc.gpsimd.tensor_add(
                            out=x2_tiles[o][:, hf * 512 : (hf + 1) * 512],
                            in0=tmp,
                            in1=x_tiles[o][:, hf * 512 : (hf + 1) * 512])
                    _bn_partial(nc, (small, statsp, dram), "bn2", o,
                                x2_tiles[o], bn2_local)

        if stop_after == "oproj":
            dump_out(x2_tiles)
            return nc

        # ======== Phase 4: BN2 + MLP ========
        with ExitStack() as s5:
            h2p = s5.enter_context(tc.tile_pool(name="h2", bufs=1))
            htp = s5.enter_context(tc.tile_pool(name="ht", bufs=1))
            outp = s5.enter_context(tc.tile_pool(name="outsb", bufs=2))
            pm1 = s5.enter_context(
                tc.tile_pool(name="pm1", bufs=2, space="PSUM"))
            pm2 = s5.enter_context(
                tc.tile_pool(name="pm2", bufs=4, space="PSUM"))

            s2v, t2v = _bn_device(nc, tc, (small, statsp, dram),
                                  x2_tiles, g2_sb, be2_sb, eps_sb, "bn2",
                                  collectives=collectives, local=bn2_local)
            # Gelu table load pinned behind rstd2 (so it can't evict the
            # Sqrt table before rstd2 runs); loads while h2/matmuls start
            warm3 = vec.tile([128, 1], F32, tag="warm3", name="warm3")
            nc.scalar.activation(out=warm3, in_=s2v[:, 0:1], func=gelu_func)

            # h2 in bf16 (fp8 here costs ~2% output error — over budget)
            h2a = h2p.tile([128, NT, TOK], BF16, tag="h2a", name="h2a")
            for t in range(NT):
                nc.vector.tensor_scalar(
                    out=h2a[:, t, :], in0=x2_tiles[t],
                    scalar1=s2v[:, t : t + 1], scalar2=t2v[:, t : t + 1],
                    op0=ALU.mult, op1=ALU.add)

            ht = htp.tile([128, NH, TOK], BF16, tag="ht", name="hts")
            for o in range(NH):
                p = pm1.tile([128, TOK], F32, tag="pm1", name="pm1s")
                for hf in range(2):
                    for k in range(NT):
                        nc.tensor.matmul(
                            p[:, hf * 512 : (hf + 1) * 512],
                            wm1_sb[:, k, o * 128 : (o + 1) * 128],
                            h2a[:, k, hf * 512 : (hf + 1) * 512],
                            start=(k == 0), stop=(k == NT - 1))
                # sim stand-in is Exp; damp its input to keep values sane
                nc.scalar.activation(
                    out=ht[:, o, :], in_=p, func=gelu_func,
                    bias=b1_sb[:, o : o + 1],
                    scale=(0.25 if sim else 1.0))

            # MLP2 in bf16 (fp8 quantization noise here lands ~3% on the
            # output — over budget; bf16 keeps it at ~0.2%).
            for o in range(NT):
                ot = outp.tile([128, TOK], F32, tag="osb", name="osbs")
                for hf in range(2):
                    p = pm2.tile([128, 512], F32, tag="pm2", name="pm2s")
                    for k in range(NH):
                        nc.tensor.matmul(
                            p,
                            wm2_sb[:, k, o * 128 : (o + 1) * 128],
                            ht[:, k, hf * 512 : (hf + 1) * 512],
                            start=(k == 0), stop=(k == NH - 1))
                    nc.vector.scalar_tensor_tensor(
                        out=ot[:, hf * 512 : (hf + 1) * 512],
                        in0=p, scalar=b2_sb[:, o : o + 1],
                        in1=x2_tiles[o][:, hf * 512 : (hf + 1) * 512],
                        op0=ALU.add, op1=ALU.add)
                    nc.sync.dma_start(out=OUT[o, :, hf * 512 : (hf + 1) * 512],
                                      in_=ot[:, hf * 512 : (hf + 1) * 512])

    return nc


_NC_CACHE = {}


def _get_nc(sim=False):
    if sim not in _NC_CACHE:
        _NC_CACHE[sim] = build(sim=sim)
    return _NC_CACHE[sim]


def make_in_maps(x, mask, Wq, bq, Wk, bk, Wv, bv, Wo, bo, g1, be1, g2, be2,
                 W1, b1m, W2, b2m):
    """Host-side sharding + layout prep. Returns list of per-core input dicts."""
    xT = np.ascontiguousarray(x.T.astype(np.float32))          # [512, 8192]
    wq = np.ascontiguousarray(
        np.asarray(Wq, np.float32).reshape(NT, 128, 512).transpose(1, 0, 2))
    wk = np.ascontiguousarray(
        np.asarray(Wk, np.float32).reshape(NT, 128, 512).transpose(1, 0, 2))
    wv = np.ascontiguousarray(
        np.asarray(Wv, np.float32).reshape(NT, 128, 512).transpose(1, 0, 2))
    # head-pair packing: partitions (j*64+d) of pair i hold Wo row (2i+j)*64+d
    # x8 prescale keeps fp8e4m3 out of its subnormal range
    fp8 = mybir.dt.np(FP8)
    wo = (np.asarray(Wo, np.float32).reshape(H // 2, 2, 64, 512)
          .transpose(1, 2, 0, 3).reshape(128, H // 2, 512) * 8.0).astype(fp8)
    wm1 = np.ascontiguousarray(
        np.asarray(W1, np.float32).reshape(NT, 128, 2048).transpose(1, 0, 2)
    ).astype(ml_dtypes.bfloat16)
    wm2 = np.ascontiguousarray(
        np.asarray(W2, np.float32).reshape(NH, 128, 512).transpose(1, 0, 2)
    ).astype(ml_dtypes.bfloat16)

    def pp(v, c):
        return np.ascontiguousarray(np.asarray(v, np.float32).reshape(c, 128).T)

    shared = {
        "wq": wq, "wk": wk, "wv": wv, "wo": wo, "wm1": wm1, "wm2": wm2,
        "bq": pp(bq, 4), "bk": pp(bk, 4), "bo": pp(bo, 4),
        "bv": np.ascontiguousarray(
            np.broadcast_to(np.asarray(bv, np.float32), (128, 512))),
        "b1m": pp(b1m, 16), "b2m": pp(b2m, 4),
        "g1": pp(g1, 4), "be1": pp(be1, 4), "g2": pp(g2, 4), "be2": pp(be2, 4),
    }
    # -4*ln2 bias: exp emits E/16 so fp8e4m3 (max 240) can't clip the tail
    am_full = np.where(np.asarray(mask, bool),
                       -4.0 * np.log(2.0), -1e9).astype(np.float32)
    in_maps = []
    for core in range(N_CORES):
        sl = xT[:, core * TOK : (core + 1) * TOK]
        m = dict(shared)
        m["xt"] = np.ascontiguousarray(sl.reshape(NT, 128, TOK))
        m["am"] = np.ascontiguousarray(am_full[core].reshape(8, 128).T)
        in_maps.append(m)
    return in_maps


_EXEC_CACHE = {}


def _get_executor():
    """Cached PJRT executor for the compiled kernel (same path
    run_bass_kernel_spmd takes under axon, but jitted once and reused)."""
    if "fn" in _EXEC_CACHE:
        return _EXEC_CACHE["fn"]
    import jax
    from jax.sharding import Mesh, PartitionSpec
    from jax.experimental.shard_map import shard_map
    import concourse.bass2jax as b2j

    nc = _get_nc(sim=False)
    b2j.install_neuronx_cc_hook()
    partition_name = (nc.partition_id_tensor.name
                      if nc.partition_id_tensor else None)
    in_names, out_names, out_avals, zero_outs = [], [], [], []
    for alloc in nc.m.functions[0].allocations:
        if not isinstance(alloc, mybir.MemoryLocationSet):
            continue
        name = alloc.memorylocations[0].name
        if alloc.kind == "ExternalInput":
            if name != partition_name:
                in_names.append(name)
        elif alloc.kind == "ExternalOutput":
            out_names.append(name)
            shape = tuple(alloc.tensor_shape)
            dtype = mybir.dt.np(alloc.dtype)
            out_avals.append(jax.core.ShapedArray(shape, dtype))
            zero_outs.append(np.zeros(shape, dtype))
    n_params = len(in_names)
    all_names = in_names + out_names
    if partition_name is not None:
        all_names = all_names + [partition_name]

    def _body(*args):
        operands = list(args)
        if partition_name is not None:
            operands.append(b2j.partition_id_tensor())
        return tuple(b2j._bass_exec_p.bind(
            *operands,
            out_avals=tuple(out_avals),
            in_names=tuple(all_names),
            out_names=tuple(out_names),
            lowering_input_output_aliases=(),
            sim_require_finite=True,
            sim_require_nnan=True,
            nc=nc,
        ))

    devices = jax.devices()[:N_CORES]
    mesh = Mesh(np.asarray(devices), ("core",))
    n_out = len(out_names)
    sharded = jax.jit(
        shard_map(_body, mesh=mesh,
                  in_specs=(PartitionSpec("core"),) * (n_params + n_out),
                  out_specs=(PartitionSpec("core"),) * n_out,
                  check_rep=False),
        keep_unused=True)

    def run(in_maps):
        per_core = [[np.asarray(m[nm]) for nm in in_names] for m in in_maps]
        concat_in = [
            np.concatenate([per_core[c][i] for c in range(N_CORES)], axis=0)
            for i in range(n_params)]
        concat_zeros = [
            np.zeros((N_CORES * z.shape[0], *z.shape[1:]), z.dtype)
            for z in zero_outs]
        out_arrs = sharded(*concat_in, *concat_zeros)
        return [
            {name: np.asarray(out_arrs[i]).reshape(
                N_CORES, *out_avals[i].shape)[c]
             for i, name in enumerate(out_names)}
            for c in range(N_CORES)]

    _EXEC_CACHE["fn"] = run
    return run


def gather_out(results):
    """results: list of per-core dicts with 'outt' [4, 128, 1024] -> [8192, 512]."""
    outs = []
    for core in range(N_CORES):
        oT = results[core]["outt"].reshape(ED, TOK)   # [512, 1024]
        outs.append(oT.T)                             # [1024, 512]
    return np.concatenate(outs, axis=0).astype(np.float32)


def kernel(**inputs) -> np.ndarray:
    inputs = dict(inputs)
    inputs.pop("b", None)
    inputs.pop("gs", None)
    in_maps = make_in_maps(**inputs)
    run = _get_executor()
    return gather_out(run(in_maps))

